# revision 58
# baseline (speedup 1.0000x reference)
"""GCCF (gnn message passing) Bass kernel for 8 trn2 NeuronCores.

Model (reference.py):
  3 layers of bipartite graph propagation:
    u_l = LReLU((user_adj @ m_{l-1} + u_{l-1}) @ Wu[l].T + 2*bu[l])
    m_l = LReLU((movie_adj @ u_{l-1} + m_{l-1}) @ Wm[l].T + 2*bm[l])
  then 100k (uid, mid) pair interactions:
    out[b] = sum_l (u_l[uid] * m_l[mid]) . wo_l + bo

Distribution (8 cores):
  - adjacency rows sharded: core c owns users [2000c, 2000c+2000) and
    movies [1000c, 1000c+1000); each core computes its slice of u_l/m_l
    against the full (all-gathered) opposite-side embedding.
  - adjacency transposed + quantized to fp8e4 ON THE HOST (scaled by
    2^13 so uniform[0,1e-3] entries land in e4m3 normal range); all 3
    layers stream A^T straight from HBM - no on-chip transpose pass.
  - propagation runs in a "scaled" domain: embeddingsT kept as
    2^13 * value so psum (scaled adj @ unscaled stat emb) adds the
    residual directly; LReLU is positively homogeneous so the scale
    passes through layers exactly. Descale (1/2^13) is folded into the
    stationary-cast and interaction-table scales.
  - stationary embeddings fp8e4 [128, kt, 64], AG'd as fp8 between
    layers -> DoubleRow fp8 matmuls (2 k-subtiles per MM).
  - interaction: per-layer tables are concatenated into [rows, 256]
    bf16 tables (u: local+wo-scaled, m: all-gathered), so each pair is
    ONE 512-byte dma_gather row per side instead of 4x 256B gathers.
    m-side gathers are desc-generated EARLY (prepare_only) against
    resident tiles and triggered right after the mcat AllGather.
  - DMA queues: sync engine carries only the A^T moving stream;
    scalar engine carries stat loads/stores + cat writes (so a stat
    load waiting on an AllGather cannot head-of-line block the stream).

Precision: fp8e4 adjacency+stats, bf16 tables -> rel_max ~ 3e-3.
"""
import os
import sys
import threading

sys.path.insert(0, "/opt/trn_rl_repo")

import numpy as np
import ml_dtypes

import concourse.bacc as bacc
import concourse.mybir as mybir
import concourse.tile as tile
from concourse.bass_utils import run_bass_kernel_spmd
from concourse.masks import make_identity

dt = mybir.dt
F32, BF16, FP8, I16 = dt.float32, dt.bfloat16, dt.float8e4, dt.int16
ALU = mybir.AluOpType
AXIS = mybir.AxisListType
ACTF = mybir.ActivationFunctionType
DR = mybir.MatmulPerfMode.DoubleRow

NCORES = 8
NU, NM, E, L, B = 16000, 8000, 64, 3, 100000
UPC, MPC = NU // NCORES, NM // NCORES        # rows per core: 2000 users, 1000 movies
UP, MP = 2048, 1024                          # padded to multiples of 512
NUCH, NMCH = UP // 512, MP // 512            # psum chunks per side (4, 2)
KUF, KUR = NM // 128, NM % 128               # u-side k-tiles: 62 full + 64 rem
KMF, KMR = NU // 128, NU % 128               # m-side k-tiles: 125 full + 0
NKU = KUF + (1 if KUR else 0)                # 63
NKM = KMF + (1 if KMR else 0)                # 125
SCALE = 8192.0                               # adj fp8 pre-scale (2^13)
CHUNK = 1024                                 # pairs per dma_gather (>=2048 wedges the DGE ring)
NCH = 14                                     # chunks per core
SC = CHUNK // 128                            # result slots per chunk (8)
NW = CHUNK // 16                             # wrapped index columns per chunk
CAP = CHUNK * NCH                            # padded pairs per core (14336)
CE = 4 * E                                   # concat width (256)
MROWS = NCORES * MP                          # combined m-table rows (8192)
USE_DR = os.environ.get("KDR", "1") == "1"   # DoubleRow fp8 matmuls
NBLK = CAP // 128                            # 128-slot blocks (112)


def _u3_kranges():
    """Static per-block k-tile ranges for the sorted-uid one-hot expansion.

    Slots are sorted by local uid with pads interleaved on a uniform grid,
    so slot s holds uid ~ s*UPC/CAP +- ~60 (DKW bound); margin 96 makes the
    static range safe for any input (host asserts coverage).
    """
    r = UPC / CAP
    rng = []
    for b in range(NBLK):
        s0 = b * 128
        lo = max(0, int(s0 * r) - 96)
        hi = min(UPC - 1, int((s0 + 127) * r) + 96)
        rng.append((lo // 128, hi // 128))
    return rng


U3KR = _u3_kranges()


def _emit(nc, tc, io):
    ctxs = []

    def pool(*a, **kw):
        p = tc.tile_pool(*a, **kw)
        ctxs.append(p)
        return p.__enter__()

    const = pool(name="const", bufs=1)
    ident_bf = const.tile([128, 128], BF16)
    make_identity(nc, ident_bf)

    wut_sb, wmt_sb, bu2_sb, bm2_sb, wo4s_sb = [], [], [], [], []
    for l in range(L):
        w = const.tile([64, 64], F32, tag=f"wut{l}")
        nc.scalar.dma_start(out=w[:], in_=io["wut"].ap()[l])
        wut_sb.append(w)
        w = const.tile([64, 64], F32, tag=f"wmt{l}")
        nc.scalar.dma_start(out=w[:], in_=io["wmt"].ap()[l])
        wmt_sb.append(w)
        bb = const.tile([64, 1], F32, tag=f"bu2{l}")
        nc.scalar.dma_start(out=bb[:], in_=io["bu2"].ap()[l])
        bu2_sb.append(bb)
        bb = const.tile([64, 1], F32, tag=f"bm2{l}")
        nc.scalar.dma_start(out=bb[:], in_=io["bm2"].ap()[l])
        bm2_sb.append(bb)
    for l in range(4):
        w = const.tile([64, 1], F32, tag=f"wo{l}")
        nc.scalar.dma_start(out=w[:], in_=io["wo4s"].ap()[l])
        wo4s_sb.append(w)
    res_sb = const.tile([128, NCH * SC], F32, tag="res")
    iot_sb = const.tile([128, 16], F32, tag="iot")
    nc.scalar.dma_start(out=iot_sb[:], in_=io["iot16"].ap())
    ones1 = const.tile([1, 128], F32, tag="ones1")
    nc.any.memset(ones1[:], 1.0)

    # ---- DRAM scratch ---------------------------------------------------
    use_shared = (not os.environ.get("KSIM")) and os.environ.get("KSHARED", "1") == "1"
    shared = "Shared" if use_shared else "Local"
    agu_in = {l: nc.dram_tensor(f"agu_in{l}", [UPC, E], FP8, kind="Internal")
              for l in (1, 2)}
    agu_out = {l: nc.dram_tensor(f"agu_out{l}", [NU, E], FP8, kind="Internal",
                                 addr_space=shared)
               for l in (1, 2)}
    agm_in = {l: nc.dram_tensor(f"agm_in{l}", [MPC, E], FP8, kind="Internal")
              for l in (1, 2)}
    agm_out = {l: nc.dram_tensor(f"agm_out{l}", [NM, E], FP8, kind="Internal",
                                 addr_space=shared)
               for l in (1, 2)}
    mcat_in = nc.dram_tensor("mcat_in", [MP, CE], BF16, kind="Internal")
    mcat_out = nc.dram_tensor("mcat_out", [MROWS, CE], BF16, kind="Internal",
                              addr_space=shared)

    # ---- pools (all coexist; ~200KB/partition total) --------------------
    gi = pool(name="gi", bufs=1)
    mgp = pool(name="mg", bufs=8)
    repp = pool(name="repsb", bufs=2)
    uidfp = pool(name="uidf", bufs=2)
    ohp = pool(name="oh", bufs=8)
    prodp = pool(name="prod", bufs=2)
    mvu = pool(name="mvu", bufs=4)
    mvm = pool(name="mvm", bufs=4)
    ustatp = pool(name="ustat", bufs=2)
    mstatp = pool(name="mstat", bufs=2)
    utp = pool(name="uT", bufs=2)
    mtp = pool(name="mT", bufs=2)
    xp = pool(name="x", bufs=2)
    ubfp = pool(name="ubf", bufs=1)
    mbfp = pool(name="mbf", bufs=1)
    s64p = pool(name="s64", bufs=2)
    catp = pool(name="cat", bufs=1)
    accp = pool(name="acc", bufs=4, space="PSUM")
    tpp = pool(name="tp", bufs=2, space="PSUM")
    xtp = pool(name="xt", bufs=2, space="PSUM")

    ucat_sb = catp.tile([128, UP // 128, CE], BF16, tag="ucat")
    mcat_sb = catp.tile([128, MP // 128, CE], BF16, tag="mcat")

    # ---- early: m-gather indices ---------------------------------------
    midx_sb = gi.tile([128, NCH * NW], I16, tag="midx")
    nc.scalar.dma_start(out=midx_sb[:], in_=io["midx"].ap())

    mgs = []

    def load_stat_u(src_ap):
        """[16000, 64] fp8 -> [128, 125, 64]; gpsimd half now, second half
        deferred (emitted at the consumer's start so a post-AG wait cannot
        head-of-line block unrelated scalar-queue work)"""
        st = ustatp.tile([128, NKM, E], FP8, tag="ustat")
        src3 = src_ap.rearrange("(a p) e -> p a e", p=128)
        h = NKM // 2
        nc.gpsimd.dma_start(out=st[:, :h, :], in_=src3[:, :h, :])

        def fin():
            nc.scalar.dma_start(out=st[:, h:, :], in_=src3[:, h:, :])
        return st, fin

    def load_stat_m(src_ap):
        """[8000, 64] fp8 -> [128, 63, 64] (62 full + 64-row remainder)"""
        st = mstatp.tile([128, NKU, E], FP8, tag="mstat")
        src3 = src_ap[: KUF * 128].rearrange("(a p) e -> p a e", p=128)
        h = KUF // 2
        nc.gpsimd.dma_start(out=st[:, :h, :], in_=src3[:, :h, :])

        def fin():
            nc.scalar.dma_start(out=st[:, h:KUF, :], in_=src3[:, h:, :])
            nc.scalar.dma_start(out=st[:KUR, KUF, :], in_=src_ap[KUF * 128:])
        return st, fin

    # ---- streamed adjacency matmul -------------------------------------
    def stream_side(adj_io, nfull, rem, stat, mvpool, width, nch, tag):
        psums = [accp.tile([64, 512], F32, tag="acc", name=f"ps{tag}{i}")
                 for i in range(nch)]
        src3 = adj_io.ap()[: nfull * 128].rearrange("(a p) n -> p a n", p=128)
        for g0 in range(0, nfull, 4):
            gn = min(4, nfull - g0)
            last = g0 + gn >= nfull
            mt = mvpool.tile([128, 4, width], FP8, tag=tag)
            nc.sync.dma_start(out=mt[:, :gn, :], in_=src3[:, g0:g0 + gn, :])
            if rem and last:
                nc.sync.dma_start(
                    out=mt[:rem, gn, :], in_=adj_io.ap()[nfull * 128:, :]
                )
            j = 0
            while j < gn:
                k = g0 + j
                if USE_DR and j + 2 <= gn:
                    for n in range(nch):
                        nc.tensor.matmul(
                            psums[n][:],
                            stat[:, k:k + 2, :],
                            mt[:, j:j + 2, n * 512:(n + 1) * 512],
                            start=(k == 0),
                            stop=(not rem and k + 2 == nfull),
                            perf_mode=DR,
                        )
                    j += 2
                else:
                    for n in range(nch):
                        nc.tensor.matmul(
                            psums[n][:],
                            stat[:, k, :],
                            mt[:, j, n * 512:(n + 1) * 512],
                            start=(k == 0),
                            stop=(not rem and k == nfull - 1),
                        )
                    j += 1
            if rem and last:
                for n in range(nch):
                    nc.tensor.matmul(
                        psums[n][:],
                        stat[:rem, nfull, :],
                        mt[:rem, gn, n * 512:(n + 1) * 512],
                        start=False,
                        stop=True,
                    )
        return psums

    # ---- epilogue: x = psum + prevT; W @ x; LReLU (scaled domain) ------
    def epilogue(psums, prevT, w_sb, b_sb, outp, width, tag):
        curT = outp.tile([64, width], F32, tag=tag)
        for n, ps in enumerate(psums):
            x = xp.tile([64, 512], F32, tag="x")
            nc.vector.tensor_tensor(
                x[:], ps[:], prevT[:, n * 512:(n + 1) * 512], ALU.add
            )
            ps2 = accp.tile([64, 512], F32, tag="acc", name=f"ps2{n}")
            nc.tensor.matmul(ps2[:], w_sb[:], x[:], start=True, stop=True)
            nc.scalar.activation(
                curT[:, n * 512:(n + 1) * 512],
                ps2[:],
                ACTF.Lrelu,
                bias=b_sb[:],
                alpha=0.01,
            )
        return curT

    # ---- concat-table column emit (scaled srcT -> bf16 slab col) -------
    def emit_cat(srcT, scale_sb, slab, nblk, col, bfpool, tag):
        tb = bfpool.tile([64, nblk * 128], BF16, tag=f"cat{tag}")
        sc = scale_sb if isinstance(scale_sb, float) else scale_sb[:]
        nc.vector.tensor_scalar_mul(tb[:], srcT[:], sc)
        for i in range(nblk):
            tp = tpp.tile([128, 128], BF16, tag="tp")
            nc.tensor.transpose(
                tp[:, :64], tb[:, i * 128:(i + 1) * 128], ident_bf[:64, :64]
            )
            nc.vector.tensor_copy(
                out=slab[:, i, col * 64:(col + 1) * 64], in_=tp[:, :64]
            )

    # ---- stationary emit: descale, transpose, fp8, DMA out for AG ------
    def emit_stat_out(srcT, dst, rows, bfpool, tag):
        nblk = (rows + 127) // 128
        tb = bfpool.tile([64, ((rows + 127) // 128) * 128], BF16, tag=f"stat{tag}")
        nc.vector.tensor_scalar_mul(tb[:], srcT[:, :nblk * 128], 1.0 / SCALE)
        for i in range(0, rows, 128):
            cw = min(128, rows - i)
            tp = tpp.tile([128, 128], BF16, tag="tp")
            nc.tensor.transpose(tp[:cw, :64], tb[:, i:i + cw], ident_bf[:64, :64])
            sbt = s64p.tile([128, 64], FP8, tag="s64")
            nc.vector.tensor_copy(out=sbt[:cw, :], in_=tp[:cw, :64])
            nc.scalar.dma_start(out=dst.ap()[i:i + cw, :], in_=sbt[:cw, :])

    def allgather(in_t, out_t):
        if os.environ.get("KSIM") or os.environ.get("KNOCC"):
            n = in_t.ap().shape[0]
            for r in range(NCORES):
                nc.scalar.dma_start(
                    out=out_t.ap()[r * n:(r + 1) * n, :], in_=in_t.ap()
                )
            return
        nc.gpsimd.collective_compute(
            "AllGather",
            ALU.bypass,
            replica_groups=[list(range(NCORES))],
            ins=[in_t.ap().opt()],
            outs=[out_t.ap().opt()],
        )

    # ---- setup ---------------------------------------------------------
    u_stats, u_fins = {}, {}
    m_stats, m_fins = {}, {}
    u_stats[0], fin = load_stat_u(io["u0stat"].ap())
    fin()
    m_stats[0], fin = load_stat_m(io["m0stat"].ap())
    fin()
    uT = utp.tile([64, UP], F32, tag="uT")
    nc.scalar.dma_start(out=uT[:], in_=io["ueT"].ap())
    mT = mtp.tile([64, MP], F32, tag="mT")
    nc.scalar.dma_start(out=mT[:], in_=io["meT"].ap())
    emit_cat(uT, wo4s_sb[0], ucat_sb, UP // 128, 0, ubfp, "u")
    emit_cat(mT, 1.0 / SCALE, mcat_sb, MP // 128, 0, mbfp, "m")

    # ---- 3 layers -------------------------------------------------------
    for l in range(L):
        def do_m(l=l):
            nonlocal mT
            if l in u_fins:
                u_fins.pop(l)()
            psums = stream_side(io["amq"], KMF, KMR, u_stats[l], mvm, MP, NMCH, "mvm")
            mT = epilogue(psums, mT, wmt_sb[l], bm2_sb[l], mtp, MP, "mT")
            emit_cat(mT, 1.0 / SCALE, mcat_sb, MP // 128, l + 1, mbfp, "m")
            if l < 2:
                emit_stat_out(mT, agm_in[l + 1], MPC, mbfp, "m")
                allgather(agm_in[l + 1], agm_out[l + 1])
                m_stats[l + 1], m_fins[l + 1] = load_stat_m(agm_out[l + 1].ap())
            else:
                nc.scalar.dma_start(
                    out=mcat_in.ap().rearrange("(a p) e -> p a e", p=128),
                    in_=mcat_sb[:],
                )
                allgather(mcat_in, mcat_out)
                for ch in range(NCH):
                    mg = mgp.tile([128, SC, CE], BF16, tag="mg", name=f"mg{ch}")
                    nc.gpsimd.dma_gather(
                        out_ap=mg[:],
                        in_ap=mcat_out.ap(),
                        idxs_ap=midx_sb[:, ch * NW:(ch + 1) * NW],
                        num_idxs=CHUNK,
                        num_idxs_reg=CHUNK,
                        elem_size=CE,
                    )
                    mgs.append(mg)

        def do_u(l=l):
            nonlocal uT
            if l in m_fins:
                m_fins.pop(l)()
            psums = stream_side(io["auq"], KUF, KUR, m_stats[l], mvu, UP, NUCH, "mvu")
            uT = epilogue(psums, uT, wut_sb[l], bu2_sb[l], utp, UP, "uT")
            emit_cat(uT, wo4s_sb[l + 1], ucat_sb, UP // 128, l + 1, ubfp, "u")
            if l < 2:
                emit_stat_out(uT, agu_in[l + 1], UPC, ubfp, "u")
                allgather(agu_in[l + 1], agu_out[l + 1])
                u_stats[l + 1], u_fins[l + 1] = load_stat_u(agu_out[l + 1].ap())

        if l == 1:
            do_u()
            do_m()
        else:
            do_m()
            do_u()

    # ---- interaction: one-hot u expansion on PE + products -------------
    if os.environ.get("KPHASE") == "layers":
        nc.any.memset(res_sb[:], 0.0)
        nc.scalar.dma_start(out=io["res"].ap(), in_=res_sb[:])
        for p in reversed(ctxs):
            p.__exit__(None, None, None)
        return

    def emit_front(ch):
        """uid replicate + one-hot planes for chunk ch (PE/ACT/DVE front)."""
        uidf = uidfp.tile([1, CHUNK], F32, tag="uidf")
        nc.scalar.dma_start(out=uidf[:], in_=io["uidf"].ap()[ch:ch + 1, :])
        rep = repp.tile([128, CHUNK], dt.float16, tag="rep")
        for h in range(2):
            xt = xtp.tile([128, 512], F32, tag="xt")
            nc.tensor.matmul(
                xt[:], ones1[:], uidf[:, h * 512:(h + 1) * 512],
                start=True, stop=True,
            )
            nc.scalar.copy(rep[:, h * 512:(h + 1) * 512], xt[:])
        cklo = min(U3KR[ch * SC + a][0] for a in range(SC))
        ckhi = max(U3KR[ch * SC + a][1] for a in range(SC))
        ohs = {}
        for k in range(cklo, ckhi + 1):
            oh = ohp.tile([128, CHUNK], FP8, tag="oh", name=f"oh{k % 8}")
            nc.vector.tensor_scalar(
                oh[:], rep[:], iot_sb[:, k:k + 1], None, ALU.is_equal
            )
            ohs[k] = oh
        return ohs

    def emit_back(ch, ohs):
        """one-hot expansion matmuls + products for chunk ch."""
        mg = mgs[ch]
        for a0 in range(0, SC, 2):
            xt = xtp.tile([128, 512], F32, tag="xt")
            for j in range(2):
                a = a0 + j
                klo, khi = U3KR[ch * SC + a]
                for k in range(klo, khi + 1):
                    nc.tensor.matmul(
                        xt[:, j * CE:(j + 1) * CE],
                        ohs[k][:, a * 128:(a + 1) * 128],
                        ucat_sb[:, k, :],
                        start=(k == klo), stop=(k == khi),
                    )
            prod = prodp.tile([128, 2, CE], F32, tag="prod")
            nc.vector.tensor_tensor(
                prod[:], xt[:].rearrange("p (a e) -> p a e", a=2),
                mg[:, a0:a0 + 2, :], ALU.mult,
            )
            nc.vector.tensor_reduce(
                res_sb[:, ch * SC + a0:ch * SC + a0 + 2], prod[:],
                axis=AXIS.X, op=ALU.add,
            )

    # software-pipelined: chunk ch+1's front stages are emitted before
    # chunk ch's back stages so engine FIFOs overlap across chunks
    pend = emit_front(0)
    for ch in range(NCH):
        nxt = emit_front(ch + 1) if ch + 1 < NCH else None
        emit_back(ch, pend)
        pend = nxt

    nc.scalar.dma_start(out=io["res"].ap(), in_=res_sb[:])

    for p in reversed(ctxs):
        p.__exit__(None, None, None)


def _build():
    ndev = 1 if os.environ.get("KSIM") else NCORES
    nc = bacc.Bacc("TRN2", num_devices=ndev, debug=False)
    io = {}
    io["auq"] = nc.dram_tensor("auq", [NM, UP], FP8, kind="ExternalInput")
    io["amq"] = nc.dram_tensor("amq", [NU, MP], FP8, kind="ExternalInput")
    io["u0stat"] = nc.dram_tensor("u0stat", [NU, E], FP8, kind="ExternalInput")
    io["m0stat"] = nc.dram_tensor("m0stat", [NM, E], FP8, kind="ExternalInput")
    io["ueT"] = nc.dram_tensor("ueT", [E, UP], F32, kind="ExternalInput")
    io["meT"] = nc.dram_tensor("meT", [E, MP], F32, kind="ExternalInput")
    io["wut"] = nc.dram_tensor("wut", [L, E, E], F32, kind="ExternalInput")
    io["wmt"] = nc.dram_tensor("wmt", [L, E, E], F32, kind="ExternalInput")
    io["bu2"] = nc.dram_tensor("bu2", [L, E, 1], F32, kind="ExternalInput")
    io["bm2"] = nc.dram_tensor("bm2", [L, E, 1], F32, kind="ExternalInput")
    io["wo4s"] = nc.dram_tensor("wo4s", [4, E, 1], F32, kind="ExternalInput")
    io["uidf"] = nc.dram_tensor("uidf", [NCH, CHUNK], F32, kind="ExternalInput")
    io["iot16"] = nc.dram_tensor("iot16", [128, 16], F32, kind="ExternalInput")
    io["midx"] = nc.dram_tensor("midx", [128, NCH * NW], I16, kind="ExternalInput")
    io["res"] = nc.dram_tensor("res", [128, NCH * SC], F32, kind="ExternalOutput")

    with tile.TileContext(nc) as tc:
        _emit(nc, tc, io)
    nc.compile()
    return nc


_cache = threading.local()


def _get_nc():
    nc = getattr(_cache, "nc", None)
    if nc is None:
        nc = _build()
        _cache.nc = nc
    return nc


def _wrap_idx(arr):
    """[CAP] int16 -> [128, NCH*NW] wrapped layout for dma_gather."""
    w = arr.reshape(NCH, NW, 16).transpose(2, 0, 1)   # [16, NCH, NW]
    w = np.tile(w, (8, 1, 1)).reshape(128, NCH * NW)
    return np.ascontiguousarray(w)


def _prep_in_maps(user_adj, movie_adj, user_emb, movie_emb, Wu, bu, Wm, bm,
                  Wo, bo, user_id, movie_id):
    E4 = ml_dtypes.float8_e4m3
    user_adj = np.asarray(user_adj, np.float32)
    movie_adj = np.asarray(movie_adj, np.float32)
    user_emb = np.asarray(user_emb, np.float32)
    movie_emb = np.asarray(movie_emb, np.float32)
    Wu, bu = np.asarray(Wu, np.float32), np.asarray(bu, np.float32)
    Wm, bm = np.asarray(Wm, np.float32), np.asarray(bm, np.float32)
    Wo, bo = np.asarray(Wo, np.float32), np.asarray(bo, np.float32)
    user_id = np.asarray(user_id, np.int32)
    movie_id = np.asarray(movie_id, np.int32)

    wo = Wo[0]                                            # [(L+1)*E]
    wut = np.ascontiguousarray(Wu.transpose(0, 2, 1))
    wmt = np.ascontiguousarray(Wm.transpose(0, 2, 1))
    bu2 = np.ascontiguousarray((2.0 * SCALE * bu).reshape(L, E, 1))
    bm2 = np.ascontiguousarray((2.0 * SCALE * bm).reshape(L, E, 1))
    wo4s = np.ascontiguousarray((wo / SCALE).reshape(4, E, 1).astype(np.float32))

    # adjacency: transpose + scale + fp8 quantize (full, then slice per core)
    auq_full = (user_adj.T * SCALE).astype(E4)            # [NM, NU]
    amq_full = (movie_adj.T * SCALE).astype(E4)           # [NU, NM]
    u0stat = user_emb.astype(E4)
    m0stat = movie_emb.astype(E4)

    # bucket pairs by uid owner
    own = user_id // UPC
    order = np.argsort(own, kind="stable")
    counts = np.bincount(own, minlength=NCORES)
    assert counts.max() <= CAP, f"bucket overflow: {counts.max()} > {CAP}"
    starts = np.zeros(NCORES + 1, np.int64)
    np.cumsum(counts, out=starts[1:])

    iot16 = np.ascontiguousarray(
        (np.arange(128)[:, None] + 128 * np.arange(16)[None, :]).astype(np.float32)
    )

    in_maps = []
    metas = []
    for c in range(NCORES):
        idx_c = order[starts[c]: starts[c + 1]]
        n_c = len(idx_c)
        npad = CAP - n_c
        # sort slots by local uid, interleaving pads on a uniform uid grid so
        # slot s holds uid ~ s*UPC/CAP (required by the static U3KR ranges)
        uid_real = (user_id[idx_c] - c * UPC).astype(np.int32)
        pad_uid = np.floor((np.arange(npad) + 0.5) * UPC / max(npad, 1)).astype(np.int32)
        all_uid = np.concatenate([uid_real, pad_uid])
        mids = movie_id[idx_c]
        mid_mapped = ((mids // MPC) * MP + (mids % MPC)).astype(np.int32)
        all_mid = np.concatenate([mid_mapped, np.zeros(npad, np.int32)])
        all_orig = np.concatenate([idx_c, np.full(npad, -1, np.int64)])
        perm = np.argsort(all_uid, kind="stable")
        uid_re = all_uid[perm]
        mid_re = all_mid[perm].astype(np.int16)
        orig = all_orig[perm]
        for b in range(NBLK):
            seg = uid_re[b * 128:(b + 1) * 128]
            klo, khi = U3KR[b]
            assert seg.min() >= klo * 128 and seg.max() <= khi * 128 + 127, (
                f"u3 range violation block {b}: [{seg.min()},{seg.max()}] "
                f"vs ktiles [{klo},{khi}]"
            )

        auq = np.zeros((NM, UP), E4)
        auq[:, :UPC] = auq_full[:, c * UPC:(c + 1) * UPC]
        amq = np.zeros((NU, MP), E4)
        amq[:, :MPC] = amq_full[:, c * MPC:(c + 1) * MPC]
        ueT = np.zeros((E, UP), np.float32)
        ueT[:, :UPC] = user_emb[c * UPC:(c + 1) * UPC].T * SCALE
        meT = np.zeros((E, MP), np.float32)
        meT[:, :MPC] = movie_emb[c * MPC:(c + 1) * MPC].T * SCALE

        in_maps.append({
            "auq": auq,
            "amq": amq,
            "u0stat": u0stat,
            "m0stat": m0stat,
            "ueT": ueT,
            "meT": meT,
            "wut": wut,
            "wmt": wmt,
            "bu2": bu2,
            "bm2": bm2,
            "wo4s": wo4s,
            "uidf": np.ascontiguousarray(uid_re.astype(np.float32).reshape(NCH, CHUNK)),
            "iot16": iot16,
            "midx": _wrap_idx(mid_re),
        })
        metas.append(orig)

    return in_maps, metas, float(bo[0])


def _postprocess(results, metas, bo0):
    out = np.zeros(B, np.float32)
    for c in range(NCORES):
        orig = metas[c]
        r = results[c]["res"]                             # [128, NCH*SC]
        vals = r.reshape(128, NCH, SC).transpose(1, 2, 0).reshape(CAP)
        mask = orig >= 0
        out[orig[mask]] = vals[mask]
    return out + np.float32(bo0)


def kernel(user_adj, movie_adj, user_emb, movie_emb, Wu, bu, Wm, bm, Wo, bo,
           user_id, movie_id):
    in_maps, metas, bo0 = _prep_in_maps(
        user_adj, movie_adj, user_emb, movie_emb, Wu, bu, Wm, bm, Wo, bo,
        user_id, movie_id,
    )
    nc = _get_nc()
    res = run_bass_kernel_spmd(nc, in_maps, core_ids=list(range(NCORES)))
    return _postprocess(res.results, metas, bo0)
